# revision 36
# baseline (speedup 1.0000x reference)
"""Trainium2 Bass kernel for nn_Attention_10582799417937 (v5: fp8 + DoubleRow).

Data-parallel over batch (32 -> 4 per core x 8 cores), weights replicated.
All matmuls except the final output projection run in fp8(e4m3); wherever the
contraction spans two 128-deep K-tiles they are merged into a single
MatmulPerfMode.DoubleRow instruction (2 k-subtiles per pass):
  - depthwise 3x3 convs on PE as diag matmuls, taps paired 2-per-instruction
    via overlapping-window APs (5 DR matmuls instead of 9 plain)
  - q/k/v pointwise projections: C=384 contraction = 1 DR (256) + 1 plain (128)
  - softmax denominators via ones-mask matmuls, jc-paired DR
  - attn@v, jc-paired DR
Final projection stays bf16 (fp8 there costs ~3% rel err: attention is a
near-uniform average here so output variation is small vs its mean).
Elementwise work: exp + y2/v evictions on ACT; y1/q/k evictions, reciprocal
and normalize on DVE; x pad-copies and memsets on DVE/GPSIMD (GPSIMD cannot
touch PSUM on TRN2). Evictions are batched (y2: one [128,768] per batch; k:
one [128,768] with the bias applied via a rank-1 K=1 matmul into the psum).
Output is written bf16 (cast to f32 on host) to halve output DMA traffic.

HAM clock-gate mitigation: the PE only holds its 2.4GHz clock when array
activity stays high, and K=64/M=64 matmuls read as half-active. So dots runs
at K=128 against two zero-padded k copies (the other head's rows zeroed), and
attn@v computes h1 first at full M=128 (rows 0:64 get garbage, different
partitions so PSUM start-overwrite is safe) before h0's M=64 DoubleRow group
overwrites rows 0:64. Identical math, measurably higher sustained clock.
"""
import sys
import numpy as np
import ml_dtypes

sys.path.insert(0, "/opt/trn_rl_repo")

import concourse.bass as bass
import concourse.mybir as mybir
import concourse.tile as tile
from concourse import bacc
from concourse.bass_utils import run_bass_kernel_spmd

# ---- problem constants (hardcoded per spec) ----
B, C, H, W = 32, 384, 32, 32
HEADS, D = 6, 64
INNER = HEADS * D          # 384
SCALE = D ** -0.5
EPS = 1e-5
N_CORES = 8
B_LOC = B // N_CORES       # 4
HW = H * W                 # 1024
HK, WK = H // 2, W // 2
JK = HK * WK               # 256
KC = C // 128              # 3 channel chunks
MC = INNER // 128          # 3 inner chunks (also head pairs)
NPAIR = HEADS // 2         # 3

BF16 = mybir.dt.bfloat16
F32 = mybir.dt.float32
FP8 = mybir.dt.float8e4
NP_FP8 = ml_dtypes.float8_e4m3
AL = mybir.AluOpType
AF = mybir.ActivationFunctionType
DR = mybir.MatmulPerfMode.DoubleRow

WS = 16.0                  # fp8 weight prescale (folded out at psum evict)
IWS = 1.0 / WS

# padded per-batch image: [34 rows x 34 cols]; data at rows 1..32, cols 1..32.
PADR = 34
PADN = PADR * PADR

# tap pairs for DoubleRow conv: (tapA, tapB) by (dy, dx); last pair reuses the
# (2,2) window with a zero diag block as subtile B (delta 0).
TAP_PAIRS = [((0, 0), (0, 1)), ((0, 2), (1, 0)), ((1, 1), (1, 2)),
             ((2, 0), (2, 1)), ((2, 2), None)]
N_TAPS = 2 * len(TAP_PAIRS)  # 10 diag blocks (incl. zero pad)


def _off(t):
    return t[0] * PADR + t[1]


def _pair_win(xp4, bb, tA, tB, n2=None, stride=1):
    """Overlapping-window rhs AP [128, 2, rows, cols] for a DoubleRow conv
    matmul: subtile dim selects tap A/B via a hand-set stride."""
    dy, dx = tA
    if stride == 1:
        w = xp4[:, bb, dy + 16 * n2:dy + 16 * n2 + 16, dx:dx + W]
    else:
        w = xp4[:, bb, dy:dy + 31:2, dx:dx + 31:2]
    u = w.unsqueeze(1)
    delta = (_off(tB) - _off(tA)) if tB is not None else 0
    u.ap[1] = [delta, 2]
    return u


def build_nc():
    nc = bacc.Bacc(None, target_bir_lowering=False)
    x_ext = nc.declare_dram_parameter("x", [B_LOC, C, H, W], FP8, False)
    aq_ext = nc.declare_dram_parameter("aq", [C, INNER], FP8, False)
    ak_ext = nc.declare_dram_parameter("ak", [C, INNER], FP8, False)
    av_ext = nc.declare_dram_parameter("av", [C, INNER], FP8, False)
    w2_ext = nc.declare_dram_parameter("w2", [INNER, C], BF16, False)
    qd_ext = nc.declare_dram_parameter("qdiag", [N_TAPS * C, 128], FP8, False)
    kd_ext = nc.declare_dram_parameter("kvdiag", [N_TAPS * C, 128], FP8, False)
    bq_ext = nc.declare_dram_parameter("bq", [INNER, 1], F32, False)
    bkr_ext = nc.declare_dram_parameter("bkr", [1, INNER], BF16, False)
    b2_ext = nc.declare_dram_parameter("b2", [C, 1], F32, False)
    out_ext = nc.declare_dram_parameter("out", [B_LOC, C, H, W], BF16, True)

    from contextlib import ExitStack
    with tile.TileContext(nc) as tc, ExitStack() as ctx:
        wpool = ctx.enter_context(tc.tile_pool(name="weights", bufs=1))
        xpool = ctx.enter_context(tc.tile_pool(name="xp", bufs=6))
        y1pool = ctx.enter_context(tc.tile_pool(name="y1", bufs=4))
        y2pool = ctx.enter_context(tc.tile_pool(name="y2", bufs=4))
        qpool = ctx.enter_context(tc.tile_pool(name="q", bufs=6))
        kpool = ctx.enter_context(tc.tile_pool(name="k", bufs=6))
        vpool = ctx.enter_context(tc.tile_pool(name="v", bufs=2))
        epool = ctx.enter_context(tc.tile_pool(name="et", bufs=12))
        opool = ctx.enter_context(tc.tile_pool(name="outT", bufs=6))
        xspool = ctx.enter_context(tc.tile_pool(name="xs", bufs=3))
        fpool = ctx.enter_context(tc.tile_pool(name="fin", bufs=3))
        rpool = ctx.enter_context(tc.tile_pool(name="recip", bufs=3))
        psbig = ctx.enter_context(tc.tile_pool(name="psbig", bufs=4, space="PSUM"))

        # ---- load weights (persistent) ----
        def wload(ext, shape, dtype, tag):
            t = wpool.tile(shape, dtype, tag=tag, name=tag)
            nc.sync.dma_start(t[:], ext[:, :])
            return t

        # full [384, INNER] as 3 kc chunks into one [128, 3*INNER] tile;
        # kc0,kc1 are adjacent so they form the DoubleRow subtile pair
        def wload3(ext, tag):
            t = wpool.tile([128, KC * INNER], FP8, tag=tag, name=tag)
            nc.sync.dma_start(
                t[:].rearrange("p (kc m) -> p kc m", kc=KC),
                ext[:, :].rearrange("(kc p) m -> p kc m", p=128))
            return t

        def dgload_tile(tag):
            return wpool.tile([128, N_TAPS * KC * 128], FP8, tag=tag, name=tag)

        def dgload_kc(t, ext, kc_):
            # per-kc slice: blocks kc_*N_TAPS .. (kc_+1)*N_TAPS
            nblk = N_TAPS * 128
            nc.sync.dma_start(
                t[:, kc_ * nblk:(kc_ + 1) * nblk].rearrange(
                    "p (blk d) -> p blk d", d=128),
                ext[kc_ * N_TAPS * 128:(kc_ + 1) * N_TAPS * 128, :].rearrange(
                    "(blk p) d -> p blk d", p=128))

        qd_t = dgload_tile("qdall")
        kd_t = dgload_tile("kdall")
        # kc0 of both paths first (gates the first conv chunk), then the rest
        dgload_kc(qd_t, qd_ext, 0)
        dgload_kc(kd_t, kd_ext, 0)
        for kc_ in (1, 2):
            dgload_kc(qd_t, qd_ext, kc_)
            dgload_kc(kd_t, kd_ext, kc_)
        qdv = qd_t[:].rearrange("p (kc t d) -> p kc t d", kc=KC, t=N_TAPS)
        kdv = kd_t[:].rearrange("p (kc t d) -> p kc t d", kc=KC, t=N_TAPS)

        aq_sb = wload3(aq_ext, "aq")
        ak_sb = wload3(ak_ext, "ak")
        av_sb = wload3(av_ext, "av")
        aqv = aq_sb[:].rearrange("p (kc m) -> p kc m", kc=KC)
        akv = ak_sb[:].rearrange("p (kc m) -> p kc m", kc=KC)
        avv = av_sb[:].rearrange("p (kc m) -> p kc m", kc=KC)

        w2_sb = wpool.tile([128, MC * C], BF16, tag="w2", name="w2")
        nc.sync.dma_start(
            w2_sb[:].rearrange("p (mc m) -> p mc m", mc=MC),
            w2_ext[:, :].rearrange("(mc p) m -> p mc m", p=128))
        w2v = w2_sb[:].rearrange("p (mc m) -> p mc m", mc=MC)

        def bload(ext, tag):
            t = wpool.tile([128, MC], F32, tag=tag, name=tag)
            nc.sync.dma_start(
                t[:].unsqueeze(2),
                ext[:, :].rearrange("(m p) o -> p m o", p=128))
            return t

        bq_sb = bload(bq_ext, "bq")
        b2_sb = bload(b2_ext, "b2")
        bkrow = wpool.tile([128, INNER], BF16, tag="bkrow", name="bkrow")
        nc.sync.dma_start(bkrow[0:1, :], bkr_ext[:, :])
        onesj = wpool.tile([128, JK], BF16, tag="onesj", name="onesj")
        nc.vector.memset(onesj[0:1, :], 1.0)

        # ones-masks for denominator matmuls (fp8), jc-paired layout [128,2,128]
        maskA = wpool.tile([128, 256], FP8, tag="maskA", name="maskA")
        maskB = wpool.tile([128, 256], FP8, tag="maskB", name="maskB")
        mAv = maskA[:].rearrange("p (s m) -> p s m", s=2)
        mBv = maskB[:].rearrange("p (s m) -> p s m", s=2)

        def mask_init():
            nc.gpsimd.memset(maskA[:], 0.0)
            nc.gpsimd.memset(mAv[:, :, 0:64], 1.0)
            nc.gpsimd.memset(maskB[:], 0.0)
            nc.gpsimd.memset(mBv[:, :, 64:128], 1.0)

        # ---- x staging: contiguous DMA then pad-copy (DVE early, gpsimd late) ----
        def xp_load(b01, eng):
            tiles = []
            for kc_ in range(KC):
                xs = xspool.tile([128, 2 * HW], FP8, tag="xs", name="xs")
                src = x_ext[2 * b01:2 * b01 + 2, kc_ * 128:(kc_ + 1) * 128, :, :]
                nc.gpsimd.dma_start(
                    xs[:].rearrange("p (b hw) -> p b hw", b=2),
                    src.rearrange("b c h w -> c b (h w)"))
                xp = xpool.tile([128, 2 * PADN], FP8, tag="xp", name="xp")
                xp4 = xp[:].rearrange("p (b r c) -> p b r c", b=2, c=PADR)
                eng.memset(xp4[:, :, 0:1, :], 0.0)
                eng.memset(xp4[:, :, 33:34, :], 0.0)
                eng.memset(xp4[:, :, 1:33, 0:1], 0.0)
                eng.memset(xp4[:, :, 1:33, 33:34], 0.0)
                eng.tensor_copy(
                    xp4[:, :, 1:33, 1:33],
                    xs[:].rearrange("p (b h w) -> p b h w", b=2, w=W))
                tiles.append(xp4)
            return tiles

        def conv_chunks(xp4s, b):
            """Depthwise convs for batch b (bb = b % 2) on PE, fp8 DoubleRow.
            Returns ((y1pair, y1last, y2pair, y2last), [chunk closures])."""
            bb = b % 2
            y1p = y1pool.tile([128, 2 * HW], FP8, tag="y1p", name="y1p")
            y1l = y1pool.tile([128, HW], FP8, tag="y1l", name="y1l")
            y2a = y2pool.tile([128, KC * JK], FP8, tag="y2a", name="y2a")

            def chunk(kc_):
                xp4 = xp4s[kc_]
                ps1 = psbig.tile([128, HW], F32, tag="ps", name="ps1")
                for n2 in range(2):
                    for pi, (tA, tB) in enumerate(TAP_PAIRS):
                        nc.tensor.matmul(
                            ps1[:, n2 * 512:(n2 + 1) * 512],
                            qdv[:, kc_, 2 * pi:2 * pi + 2, :],
                            _pair_win(xp4, bb, tA, tB, n2=n2, stride=1),
                            start=(pi == 0), stop=(pi == len(TAP_PAIRS) - 1),
                            perf_mode=DR)
                dst1 = y1p[:, kc_ * HW:(kc_ + 1) * HW] if kc_ < 2 else y1l[:]
                nc.vector.tensor_scalar_mul(dst1, ps1[:], IWS)
                psf2 = psbig.tile([128, HW], F32, tag="ps", name="ps2")
                ps2 = psf2[:, 0:JK]
                for pi, (tA, tB) in enumerate(TAP_PAIRS):
                    nc.tensor.matmul(
                        ps2,
                        kdv[:, kc_, 2 * pi:2 * pi + 2, :],
                        _pair_win(xp4, bb, tA, tB, stride=2),
                        start=(pi == 0), stop=(pi == len(TAP_PAIRS) - 1),
                        perf_mode=DR)
                nc.scalar.mul(y2a[:, kc_ * JK:(kc_ + 1) * JK], ps2, IWS)

            res = (y1p[:].rearrange("p (s n) -> p s n", s=2), y1l[:],
                   y2a[:, 0:2 * JK].rearrange("p (s n) -> p s n", s=2),
                   y2a[:, 2 * JK:3 * JK])
            return res, [lambda kc_=kc_: chunk(kc_) for kc_ in range(KC)]

        def ab_phase(b, y1p, y1l, y2p, y2l):
            """Pointwise projections -> q [3][128,1024], k [3][128,256],
            vT2 [128, 2, 384] (jc-paired), all fp8."""
            q_sb = []
            for mc_ in range(MC):
                qt = qpool.tile([128, HW], FP8, tag="q", name="qsb")
                ps = psbig.tile([128, HW], F32, tag="ps", name="psA")
                for n2 in range(2):
                    sl = slice(n2 * 512, (n2 + 1) * 512)
                    nc.tensor.matmul(
                        ps[:, sl], aqv[:, 0:2, mc_ * 128:(mc_ + 1) * 128],
                        y1p[:, :, sl], start=True, stop=False, perf_mode=DR)
                for n2 in range(2):
                    sl = slice(n2 * 512, (n2 + 1) * 512)
                    nc.tensor.matmul(
                        ps[:, sl], aqv[:, 2, mc_ * 128:(mc_ + 1) * 128],
                        y1l[:, sl], start=False, stop=True)
                nc.vector.tensor_scalar(qt[:], ps[:], IWS, bq_sb[:, mc_:mc_ + 1],
                                        AL.mult, AL.add)
                q_sb.append(qt)

            # k is stored twice with the other head's rows zeroed, so the
            # dots matmuls run at K=128 (full PE-array activity for HAM)
            ka0 = kpool.tile([128, MC * JK], FP8, tag="ka0", name="ka0")
            ka1 = kpool.tile([128, MC * JK], FP8, tag="ka1", name="ka1")
            nc.gpsimd.memset(ka0[64:128, :], 0.0)
            nc.gpsimd.memset(ka1[0:64, :], 0.0)
            psk = psbig.tile([128, HW], F32, tag="ps", name="psBk")
            for mc_ in range(MC):
                ps = psk[:, mc_ * JK:(mc_ + 1) * JK]
                nc.tensor.matmul(
                    ps, bkrow[0:1, mc_ * 128:(mc_ + 1) * 128],
                    onesj[0:1, :], start=True, stop=False)
                nc.tensor.matmul(
                    ps, akv[:, 0:2, mc_ * 128:(mc_ + 1) * 128],
                    y2p[:], start=False, stop=False, perf_mode=DR)
                nc.tensor.matmul(
                    ps, akv[:, 2, mc_ * 128:(mc_ + 1) * 128],
                    y2l[:], start=False, stop=True)
            nc.vector.tensor_scalar_mul(ka0[0:64, :], psk[0:64, 0:MC * JK], IWS)
            nc.vector.tensor_scalar_mul(ka1[64:128, :],
                                        psk[64:128, 0:MC * JK], IWS)
            k_sb = [(ka0[:, p * JK:(p + 1) * JK],
                     ka1[:, p * JK:(p + 1) * JK]) for p in range(MC)]

            vt = vpool.tile([128, 2 * INNER], FP8, tag="v", name="vsb")
            vt2 = vt[:].rearrange("p (s m) -> p s m", s=2)
            av2 = av_sb[:][:, 0:2 * INNER].rearrange("p (s m) -> p s m", s=2)
            for jc in range(2):
                psf = psbig.tile([128, HW], F32, tag="ps", name="psBv")
                ps = psf[:, 0:INNER]
                y2pj = y2p[:, :, jc * 128:(jc + 1) * 128]
                nc.tensor.matmul(ps, y2pj, av2, start=True, stop=False,
                                 perf_mode=DR)
                nc.tensor.matmul(ps, y2l[:, jc * 128:(jc + 1) * 128],
                                 avv[:, 2, :], start=False, stop=True)
                nc.scalar.mul(vt2[:, jc, :], ps, IWS)
            return q_sb, k_sb, vt2

        def dots_chunks(b, q_sb, k_sb):
            """dots^T + exp -> e tiles [pair][h01] = [128, 2, 1024] fp8."""
            et = [[None, None] for _ in range(NPAIR)]

            def chunk(p):
                e0 = epool.tile([128, 2 * HW], FP8, tag="et", name="et")
                e1 = epool.tile([128, 2 * HW], FP8, tag="et", name="et")
                for jc in range(2):
                    # h01=0 on PE rows 0:64, h01=1 on rows 64:128 -- emitted
                    # interleaved so the two row groups execute concurrently
                    psd0 = psbig.tile([128, HW], F32, tag="ps", name="psd0")
                    psd1 = psbig.tile([128, HW], F32, tag="ps", name="psd1")
                    for ic in range(2):
                        nc.tensor.matmul(
                            psd0[:, ic * 512:(ic + 1) * 512],
                            k_sb[p][0][:, jc * 128:(jc + 1) * 128],
                            q_sb[p][:, ic * 512:(ic + 1) * 512],
                            start=True, stop=True)
                        nc.tensor.matmul(
                            psd1[:, ic * 512:(ic + 1) * 512],
                            k_sb[p][1][:, jc * 128:(jc + 1) * 128],
                            q_sb[p][:, ic * 512:(ic + 1) * 512],
                            start=True, stop=True)
                    nc.scalar.activation(
                        e0[:, jc * HW:(jc + 1) * HW], psd0[:], AF.Exp,
                        scale=SCALE)
                    nc.scalar.activation(
                        e1[:, jc * HW:(jc + 1) * HW], psd1[:], AF.Exp,
                        scale=SCALE)
                et[p][0] = e0[:].rearrange("p (jc i) -> p jc i", jc=2)
                et[p][1] = e1[:].rearrange("p (jc i) -> p jc i", jc=2)

            return et, [lambda p=p: chunk(p) for p in range(NPAIR)]

        def denav_chunks(b, et, vt2):
            """Denominators (mask DR matmuls) + attn@v (DR) + normalize."""
            ots = []

            def chunk(p):
                psn = psbig.tile([128, HW], F32, tag="ps", name="psn")
                for ic in range(2):
                    sl = slice(ic * 512, (ic + 1) * 512)
                    nc.tensor.matmul(psn[:, sl], mAv, et[p][0][:, :, sl],
                                     start=True, stop=False, perf_mode=DR)
                    nc.tensor.matmul(psn[:, sl], mBv, et[p][1][:, :, sl],
                                     start=False, stop=True, perf_mode=DR)

                pso = psbig.tile([128, HW], F32, tag="ps", name="pso")
                # h01=1 first at FULL M=128 (rows 0:64 get garbage =
                # v_h0^T e_h1, full PE-column activity for HAM); h01=0 then
                # overwrites rows 0:64 with its own accumulation group.
                for ic in range(2):
                    for jc in range(2):
                        nc.tensor.matmul(
                            pso[:, ic * 512:(ic + 1) * 512],
                            vt2[:, jc, p * 128:(p + 1) * 128],
                            et[p][1][:, jc, ic * 512:(ic + 1) * 512],
                            start=(jc == 0), stop=(jc == 1))
                for ic in range(2):
                    nc.tensor.matmul(
                        pso[0:64, ic * 512:(ic + 1) * 512],
                        vt2[:, :, p * 128:p * 128 + 64],
                        et[p][0][:, :, ic * 512:(ic + 1) * 512],
                        start=True, stop=True, perf_mode=DR,
                        tile_position=(0, 0))
                rec = rpool.tile([128, HW], F32, tag="recip", name="recip")
                nc.vector.reciprocal_approx_fast(out=rec[:], in_=psn[:])
                ot = opool.tile([128, HW], BF16, tag="outT", name="outT")
                nc.vector.tensor_tensor(ot[:], pso[:], rec[:], AL.mult)
                ots.append(ot)

            return ots, [lambda p=p: chunk(p) for p in range(NPAIR)]

        def out_chunks(b, ots):
            def chunk(mc_):
                fin = fpool.tile([128, HW], BF16, tag="fin", name="fin")
                ps = psbig.tile([128, HW], F32, tag="ps", name="psE")
                for n2 in range(2):
                    for p in range(NPAIR):
                        nc.tensor.matmul(
                            ps[:, n2 * 512:(n2 + 1) * 512],
                            w2v[:, p, mc_ * 128:(mc_ + 1) * 128],
                            ots[p][:, n2 * 512:(n2 + 1) * 512],
                            start=(p == 0), stop=(p == NPAIR - 1))
                if (b + mc_) % 2 == 0:
                    nc.scalar.activation(fin[:], ps[:], AF.Identity,
                                         bias=b2_sb[:, mc_:mc_ + 1], scale=1.0)
                else:
                    nc.vector.tensor_scalar(fin[:], ps[:], 1.0,
                                            b2_sb[:, mc_:mc_ + 1],
                                            AL.mult, AL.add)
                nc.sync.dma_start(
                    out_ext[b, mc_ * 128:(mc_ + 1) * 128, :, :],
                    fin[:].rearrange("p (h w) -> p h w", w=W))

            return [lambda mc_=mc_: chunk(mc_) for mc_ in range(MC)]

        def ilv(*seqs):
            """Round-robin interleave chunk lists."""
            seqs = [list(s) for s in seqs]
            while any(seqs):
                for s in seqs:
                    if s:
                        s.pop(0)()

        # ---- schedule ---- (fine-grained interleave: conv chunks fill the
        # exp/eviction stalls of the denav chunks; dots of the two in-flight
        # batches alternate so exp completion tracks denav consumption)
        xp0 = xp_load(0, nc.vector)
        mask_init()
        xp1 = xp_load(1, nc.gpsimd)
        y0, c0 = conv_chunks(xp0, 0)
        y1_, c1 = conv_chunks(xp0, 1)
        ilv(c0)
        ilv(c1)
        qkv0 = ab_phase(0, *y0)
        qkv1 = ab_phase(1, *y1_)
        e0, d0 = dots_chunks(0, qkv0[0], qkv0[1])
        e1, d1 = dots_chunks(1, qkv1[0], qkv1[1])
        ilv(d0, d1)
        y2_, c2 = conv_chunks(xp1, 2)
        y3_, c3 = conv_chunks(xp1, 3)
        o0, n0 = denav_chunks(0, e0, qkv0[2])
        o1, n1 = denav_chunks(1, e1, qkv1[2])
        ilv(n0, c2)
        ilv(n1, c3)
        f0 = out_chunks(0, o0)
        ilv(f0)
        qkv2 = ab_phase(2, *y2_)
        f1 = out_chunks(1, o1)
        ilv(f1)
        qkv3 = ab_phase(3, *y3_)
        e2, d2 = dots_chunks(2, qkv2[0], qkv2[1])
        e3, d3 = dots_chunks(3, qkv3[0], qkv3[1])
        ilv(d2, d3)
        o2, n2c = denav_chunks(2, e2, qkv2[2])
        o3, n3c = denav_chunks(3, e3, qkv3[2])
        ilv(n2c)
        f2 = out_chunks(2, o2)
        ilv(n3c, f2)
        f3 = out_chunks(3, o3)
        ilv(f3)

    nc.compile()
    return nc


_NC_CACHE = None


def _get_nc():
    global _NC_CACHE
    if _NC_CACHE is None:
        _NC_CACHE = build_nc()
    return _NC_CACHE


def _prep_host(inputs):
    """Fold BN into pointwise weights; fold v-bias into final bias."""
    f32 = np.float32
    bf16 = ml_dtypes.bfloat16
    inv_q = (inputs['q_gamma'] / np.sqrt(inputs['q_var'] + EPS)).astype(f32)
    sh_q = (inputs['q_beta'] - inputs['q_mean'] * inv_q).astype(f32)
    A_q = (inputs['q_pw'] * inv_q[None, :]).astype(f32)
    b_q = (inputs['q_pw'].astype(f32) @ sh_q).astype(f32)

    inv_kv = (inputs['kv_gamma'] / np.sqrt(inputs['kv_var'] + EPS)).astype(f32)
    sh_kv = (inputs['kv_beta'] - inputs['kv_mean'] * inv_kv).astype(f32)
    A_kv = (inputs['kv_pw'] * inv_kv[None, :]).astype(f32)
    b_kv = (inputs['kv_pw'].astype(f32) @ sh_kv).astype(f32)
    A_k, A_v = A_kv[:INNER], A_kv[INNER:]
    b_k, b_v = b_kv[:INNER], b_kv[INNER:]

    W2 = inputs['out_w'].astype(f32)
    b2 = (inputs['out_b'].astype(f32) + W2 @ b_v).astype(f32)

    # 10 diag blocks per (kc): tap pairs in TAP_PAIRS order, zero block pads
    def diag_blocks(taps):
        out = np.zeros((N_TAPS * C, 128), f32)
        for pi, (tA, tB) in enumerate(TAP_PAIRS):
            for si, t in enumerate((tA, tB)):
                if t is None:
                    continue
                ti = t[0] * 3 + t[1]
                blk_i = 2 * pi + si
                for kc_ in range(KC):
                    blk = np.diag(taps[kc_ * 128:(kc_ + 1) * 128, ti]) * WS
                    r0 = kc_ * N_TAPS * 128 + blk_i * 128
                    out[r0:r0 + 128, :] = blk
        return out

    qdiag = diag_blocks(inputs['q_dw'].reshape(C, 9).astype(f32))
    kvdiag = diag_blocks(inputs['kv_dw'].reshape(C, 9).astype(f32))

    return {
        'qdiag': qdiag.astype(NP_FP8),
        'kvdiag': kvdiag.astype(NP_FP8),
        'aq': np.ascontiguousarray(A_q.T * WS).astype(NP_FP8),
        'ak': np.ascontiguousarray(A_k.T * WS).astype(NP_FP8),
        'av': np.ascontiguousarray(A_v.T * WS).astype(NP_FP8),
        'w2': np.ascontiguousarray(W2.T).astype(bf16),
        'bq': b_q.reshape(INNER, 1),
        'bkr': np.ascontiguousarray((b_k * WS).reshape(1, INNER)).astype(bf16),
        'b2': b2.reshape(C, 1),
    }


def _make_in_maps(inputs):
    wmap = _prep_host(inputs)
    x8 = inputs['x'].astype(NP_FP8)
    in_maps = []
    for c in range(N_CORES):
        m = dict(wmap)
        m['x'] = np.ascontiguousarray(x8[c * B_LOC:(c + 1) * B_LOC])
        in_maps.append(m)
    return in_maps


def kernel(**inputs):
    inputs = {k: np.asarray(v) for k, v in inputs.items()}
    nc = _get_nc()
    in_maps = _make_in_maps(inputs)
    res = run_bass_kernel_spmd(nc, in_maps, core_ids=list(range(N_CORES)))
    shards = [res.results[i]['out'] for i in range(N_CORES)]
    return np.concatenate(shards, axis=0).astype(np.float32)
